# revision 54
# baseline (speedup 1.0000x reference)
import contextlib

import ml_dtypes
import numpy as np

import concourse.bacc as bacc
import concourse.bass as bass
import concourse.tile as tile
from concourse import mybir
from concourse.bass_utils import run_bass_kernel_spmd
from concourse.masks import make_identity

F32 = mybir.dt.float32
BF16 = mybir.dt.bfloat16
ACTF = mybir.ActivationFunctionType

N_CORES = 8
B, T, J, D = 32, 512, 64, 512
BPC = B // N_CORES
NT = T // 128
NK = D // 128
GD = 4 * D
ND = NT * D


def build_kernel(loop_reps=None, dma_only=False, no_store=False):
    nc = bacc.Bacc()

    ctx_d = nc.dram_tensor("c_resh", [BPC, 128, NT, D], BF16, kind="ExternalInput")
    ct_d = nc.dram_tensor("ct_resh", [BPC, 128, NK, T], BF16, kind="ExternalInput")
    qry_d = nc.dram_tensor("embd_query", [BPC, J, D], BF16, kind="ExternalInput")
    qryt_d = nc.dram_tensor("qt_resh", [BPC, 128, NK * J], BF16, kind="ExternalInput")
    wt_d = nc.dram_tensor("w_resh", [128, 12], F32, kind="ExternalInput")
    out_d = nc.dram_tensor("g_out", [BPC, 128, 3, NT, D], BF16, kind="ExternalOutput")

    with tile.TileContext(nc) as tc:
        with (
            tc.tile_pool(name="singles", bufs=1) as singles,
            tc.tile_pool(name="gpool", bufs=4) as gpool,
            tc.tile_pool(name="spool", bufs=4) as spool,
            tc.tile_pool(name="small", bufs=8) as small,
            tc.tile_pool(name="ps_trans", bufs=1, space="PSUM") as ps_trans,
            tc.tile_pool(name="ps_pall", bufs=1, space="PSUM") as ps_pall,
            tc.tile_pool(name="ps_s", bufs=1, space="PSUM") as ps_s,
            tc.tile_pool(name="ps_mm", bufs=3, space="PSUM") as ps_mm,
            tc.tile_pool(name="ps_q2c", bufs=1, space="PSUM") as ps_q2c,
            tc.tile_pool(name="ps_se", bufs=1, space="PSUM") as ps_se,
        ):
            ident = singles.tile([128, 128], F32)
            make_identity(nc, ident)
            ident_bf = singles.tile([128, 128], BF16)
            nc.vector.tensor_copy(ident_bf, ident)
            ones_row_bf = singles.tile([1, 128], BF16)
            nc.vector.memset(ones_row_bf, 1.0)
            ones_col_bf = singles.tile([128, 1], BF16)
            nc.vector.memset(ones_col_bf, 1.0)
            wt_sb = singles.tile([128, 12], F32)
            nc.scalar.dma_start(out=wt_sb, in_=wt_d[:, :])
            wt_bf = singles.tile([128, 12], BF16)
            nc.vector.tensor_copy(wt_bf, wt_sb)

            warm_ps = ps_mm.tile([128, 128], F32, tag="mm", name="warm")
            for _ in range(12):
                nc.tensor.matmul(warm_ps, lhsT=ident_bf, rhs=ident_bf,
                                 start=True, stop=True)

            loop_cm = (
                tc.For_i(0, loop_reps, 1)
                if loop_reps is not None
                else contextlib.nullcontext()
            )
            with loop_cm:
                st = [dict() for _ in range(BPC)]

                def gs(b, s, i):
                    g = st[b]["g"]
                    off = s * ND + i * D
                    return g[:, off : off + D]

                def stage_a(b):
                    v = st[b]
                    v["g"] = gpool.tile([128, 4 * ND], BF16, tag="g", name=f"g{b}")
                    cdst = v["g"][:, 0:ND].rearrange("p (i d) -> p i d", i=NT)
                    nc.scalar.dma_start(out=cdst, in_=ctx_d[b])
                    v["q"] = spool.tile([J, D], BF16, tag="q", name=f"q{b}")
                    nc.gpsimd.dma_start(out=v["q"], in_=qry_d[b])
                    v["qT"] = spool.tile([128, NK * J], BF16, tag="qt", name=f"qT{b}")
                    nc.gpsimd.dma_start(out=v["qT"], in_=qryt_d[b])
                    v["ct"] = spool.tile([128, NK * T], BF16, tag="ct", name=f"ct_sb{b}")
                    nc.scalar.dma_start(
                        out=v["ct"].rearrange("p (k t) -> p k t", k=NK), in_=ct_d[b]
                    )

                def stage_b(b):
                    v = st[b]
                    qT_sb, q_sb = v["qT"], v["q"]
                    qhatT = spool.tile([128, NK * J], BF16, tag="qhat", name=f"qhat{b}")
                    for k in range(NK):
                        nc.scalar.activation(
                            qhatT[:, J * k : J * (k + 1)],
                            qT_sb[:, J * k : J * (k + 1)],
                            ACTF.Identity,
                            bias=wt_sb[:, k : k + 1],
                            scale=wt_sb[:, 8 + k : 9 + k],
                        )
                    qt_ps2 = ps_s.tile([J, 1], F32, tag="s", name=f"qt_ps{b}")
                    for k in range(NK):
                        nc.tensor.matmul(
                            qt_ps2,
                            lhsT=qT_sb[:, J * k : J * (k + 1)],
                            rhs=wt_bf[:, 4 + k : 5 + k],
                            start=(k == 0),
                            stop=(k == NK - 1),
                        )
                    qt_col = small.tile([J, 1], F32, tag="qtc", name=f"qt_col{b}")
                    nc.scalar.copy(qt_col, qt_ps2)

                    ct_sb = v["ct"]

                    st_ps = ps_s.tile([J, T], F32, tag="s", name=f"st_ps{b}")
                    for k in range(NK):
                        nc.tensor.matmul(
                            st_ps,
                            lhsT=qhatT[:, J * k : J * (k + 1)],
                            rhs=ct_sb[:, T * k : T * (k + 1)],
                            start=(k == 0),
                            stop=(k == NK - 1),
                        )
                    ptr_sb = spool.tile([J, T], BF16, tag="pt", name=f"ptr{b}")
                    nc.scalar.activation(
                        ptr_sb, st_ps, ACTF.Exp, bias=qt_col, scale=1.0
                    )
                    v["ptr"] = ptr_sb
                    pall_ps = ps_pall.tile([128, NT * J], BF16, tag="pall", name=f"pall{b}")
                    for i in range(NT):
                        nc.tensor.transpose(
                            pall_ps[:, J * i : J * (i + 1)],
                            ptr_sb[:, 128 * i : 128 * (i + 1)],
                            ident_bf[:J, :J],
                        )
                    pall_v = pall_ps.rearrange("p (i j) -> p i j", i=NT)
                    e_sb = small.tile([128, NT], BF16, tag="e", name=f"e{b}")
                    rs_sb = small.tile([128, NT], F32, tag="rs", name=f"rs{b}")
                    recip = small.tile([128, NT], F32, tag="rcp", name=f"recip{b}")
                    nc.vector.reduce_max(e_sb, pall_v, axis=mybir.AxisListType.X)
                    nc.vector.reduce_sum(rs_sb, pall_v, axis=mybir.AxisListType.X)
                    nc.vector.reciprocal(recip, rs_sb)
                    v["e"], v["recip"] = e_sb, recip

                def stage_c(b):
                    v = st[b]
                    q_sb, ptr_sb, e_sb, recip = v["q"], v["ptr"], v["e"], v["recip"]
                    q2c_ps = ps_q2c.tile([1, D], F32, tag="q2c", name=f"q2c_ps{b}")
                    se_ps = ps_se.tile([1, 1], F32, tag="se", name=f"se_ps{b}")
                    for i in range(NT):
                        c2q_ps = ps_mm.tile([128, D], F32, tag="mm", name=f"c2q{b}_{i}")
                        nc.tensor.matmul(
                            c2q_ps,
                            lhsT=ptr_sb[:, 128 * i : 128 * (i + 1)],
                            rhs=q_sb,
                            start=True,
                            stop=True,
                        )
                        if i % 2 == 0:
                            nc.scalar.activation(
                                gs(b, 1, i),
                                c2q_ps,
                                ACTF.Copy,
                                scale=recip[:, i : i + 1],
                            )
                        else:
                            nc.vector.tensor_scalar_mul(
                                gs(b, 1, i), c2q_ps, recip[:, i : i + 1]
                            )
                        eng = nc.gpsimd if i == 0 else nc.vector
                        eng.tensor_mul(gs(b, 2, i), gs(b, 1, i), gs(b, 0, i))
                        nc.tensor.matmul(
                            q2c_ps,
                            lhsT=e_sb[:, i : i + 1],
                            rhs=gs(b, 0, i),
                            start=(i == 0),
                            stop=(i == NT - 1),
                        )
                        nc.tensor.matmul(
                            se_ps,
                            lhsT=e_sb[:, i : i + 1],
                            rhs=ones_col_bf,
                            start=(i == 0),
                            stop=(i == NT - 1),
                        )
                    if not no_store:
                        nc.sync.dma_start(
                            out=out_d[b, :, 0:2],
                            in_=v["g"][:, ND : 3 * ND].rearrange(
                                "p (s i d) -> p s i d", s=2, i=NT
                            ),
                        )
                    rcp_s = small.tile([1, 1], F32, tag="rcps", name=f"rcp_s{b}")
                    nc.vector.reciprocal(rcp_s, se_ps)
                    q2c_row = small.tile([1, D], BF16, tag="q2cr", name=f"q2c_row{b}")
                    nc.scalar.activation(
                        q2c_row, q2c_ps, ACTF.Copy, scale=rcp_s
                    )
                    bc_ps = ps_mm.tile([128, D], F32, tag="mm", name=f"bc_ps{b}")
                    nc.tensor.matmul(
                        bc_ps, lhsT=ones_row_bf, rhs=q2c_row, start=True, stop=True
                    )
                    bc_sb = small.tile([128, D], BF16, tag="bc", name=f"bc_sb{b}")
                    nc.scalar.copy(bc_sb, bc_ps)
                    for i in range(NT):
                        eng = nc.gpsimd if i in (0, 2) else nc.vector
                        eng.tensor_mul(gs(b, 3, i), gs(b, 0, i), bc_sb)
                    if not no_store:
                        nc.sync.dma_start(
                            out=out_d[b, :, 2],
                            in_=v["g"][:, 3 * ND : 4 * ND].rearrange(
                                "p (i d) -> p i d", i=NT
                            ),
                        )

                if dma_only:
                    for b in range(BPC):
                        stage_a(b)
                        v = st[b]
                        nc.sync.dma_start(
                            out=out_d[b, :, 0:2],
                            in_=v["g"][:, 0 : 2 * ND].rearrange(
                                "p (s i d) -> p s i d", s=2, i=NT
                            ),
                        )
                        nc.scalar.dma_start(
                            out=out_d[b, :, 2],
                            in_=v["g"][:, 0:ND].rearrange("p (i d) -> p i d", i=NT),
                        )
                else:
                    plan = []
                    for b in range(BPC + 2):
                        if b < BPC:
                            plan.append(("A", b))
                        if 1 <= b < BPC + 1:
                            plan.append(("B", b - 1))
                        if b >= 2:
                            plan.append(("C", b - 2))
                    for s, b in plan:
                        {"A": stage_a, "B": stage_b, "C": stage_c}[s](b)

    nc.compile()
    return nc


_NC_CACHE = None


def _get_nc():
    global _NC_CACHE
    if _NC_CACHE is None:
        _NC_CACHE = build_kernel()
    return _NC_CACHE


def _prep_in_maps(embd_context, embd_query, W):
    bf = ml_dtypes.bfloat16
    c_bf = np.asarray(embd_context, dtype=np.float32).astype(bf)
    q_bf = np.asarray(embd_query, dtype=np.float32).astype(bf)
    c_resh = np.ascontiguousarray(
        c_bf.reshape(B, NT, 128, D).transpose(0, 2, 1, 3)
    )
    ct_resh = np.ascontiguousarray(
        c_bf.reshape(B, T, NK, 128).transpose(0, 3, 2, 1)
    )
    qt_resh = np.ascontiguousarray(
        q_bf.transpose(0, 2, 1)
        .reshape(B, NK, 128, J)
        .transpose(0, 2, 1, 3)
        .reshape(B, 128, NK * J)
    )
    w_resh = np.ascontiguousarray(
        np.asarray(W, dtype=np.float32).reshape(12, 128).T
    )
    in_maps = []
    for c in range(N_CORES):
        sl = slice(c * BPC, (c + 1) * BPC)
        in_maps.append(
            {
                "c_resh": np.ascontiguousarray(c_resh[sl]),
                "ct_resh": np.ascontiguousarray(ct_resh[sl]),
                "embd_query": np.ascontiguousarray(q_bf[sl]),
                "qt_resh": np.ascontiguousarray(qt_resh[sl]),
                "w_resh": w_resh,
            }
        )
    return in_maps


def run_spmd(embd_context, embd_query, W, **spmd_kwargs):
    nc = _get_nc()
    in_maps = _prep_in_maps(embd_context, embd_query, W)
    res = run_bass_kernel_spmd(nc, in_maps, core_ids=list(range(N_CORES)), **spmd_kwargs)
    out = np.empty((B, T, GD), np.float32)
    out[:, :, :D] = np.asarray(embd_context, np.float32)
    out[:, :, D:] = np.concatenate(
        [
            res.results[c]["g_out"].transpose(0, 3, 1, 2, 4).reshape(BPC, T, 3 * D)
            for c in range(N_CORES)
        ],
        axis=0,
    ).astype(np.float32)
    return out, res


def kernel(embd_context, embd_query, W):
    out, _ = run_spmd(embd_context, embd_query, W)
    return out


# revision 55
# speedup vs baseline: 1.0624x; 1.0624x over previous
import contextlib

import ml_dtypes
import numpy as np

import concourse.bacc as bacc
import concourse.bass as bass
import concourse.tile as tile
from concourse import mybir
from concourse.bass_utils import run_bass_kernel_spmd
from concourse.masks import make_identity

F32 = mybir.dt.float32
BF16 = mybir.dt.bfloat16
ACTF = mybir.ActivationFunctionType

N_CORES = 8
B, T, J, D = 32, 512, 64, 512
BPC = B // N_CORES
NT = T // 128
NK = D // 128
GD = 4 * D
ND = NT * D


def build_kernel(loop_reps=None, dma_only=False, no_store=False):
    nc = bacc.Bacc()

    ctx_d = nc.dram_tensor("c_resh", [BPC, 128, NT, D], BF16, kind="ExternalInput")
    qry_d = nc.dram_tensor("embd_query", [BPC, J, D], BF16, kind="ExternalInput")
    qryt_d = nc.dram_tensor("qt_resh", [BPC, 128, NK * J], BF16, kind="ExternalInput")
    wt_d = nc.dram_tensor("w_resh", [128, 12], F32, kind="ExternalInput")
    out_d = nc.dram_tensor("g_out", [BPC, 128, 3, NT, D], BF16, kind="ExternalOutput")

    with tile.TileContext(nc) as tc:
        with (
            tc.tile_pool(name="singles", bufs=1) as singles,
            tc.tile_pool(name="gpool", bufs=4) as gpool,
            tc.tile_pool(name="spool", bufs=4) as spool,
            tc.tile_pool(name="small", bufs=8) as small,
            tc.tile_pool(name="ps_trans", bufs=1, space="PSUM") as ps_trans,
            tc.tile_pool(name="ps_pall", bufs=1, space="PSUM") as ps_pall,
            tc.tile_pool(name="ps_s", bufs=1, space="PSUM") as ps_s,
            tc.tile_pool(name="ps_mm", bufs=2, space="PSUM") as ps_mm,
            tc.tile_pool(name="ps_q2c", bufs=1, space="PSUM") as ps_q2c,
            tc.tile_pool(name="ps_se", bufs=1, space="PSUM") as ps_se,
        ):
            ident = singles.tile([128, 128], F32)
            make_identity(nc, ident)
            ident_bf = singles.tile([128, 128], BF16)
            nc.vector.tensor_copy(ident_bf, ident)
            ones_row_bf = singles.tile([1, 128], BF16)
            nc.vector.memset(ones_row_bf, 1.0)
            ones_col_bf = singles.tile([128, 1], BF16)
            nc.vector.memset(ones_col_bf, 1.0)
            wt_sb = singles.tile([128, 12], F32)
            nc.scalar.dma_start(out=wt_sb, in_=wt_d[:, :])
            wt_bf = singles.tile([128, 12], BF16)
            nc.vector.tensor_copy(wt_bf, wt_sb)

            warm_ps = ps_mm.tile([128, 128], F32, tag="mm", name="warm")
            for _ in range(12):
                nc.tensor.matmul(warm_ps, lhsT=ident_bf, rhs=ident_bf,
                                 start=True, stop=True)

            loop_cm = (
                tc.For_i(0, loop_reps, 1)
                if loop_reps is not None
                else contextlib.nullcontext()
            )
            with loop_cm:
                st = [dict() for _ in range(BPC)]

                def gs(b, s, i):
                    g = st[b]["g"]
                    off = s * ND + i * D
                    return g[:, off : off + D]

                def stage_a(b):
                    v = st[b]
                    v["g"] = gpool.tile([128, 4 * ND], BF16, tag="g", name=f"g{b}")
                    cdst = v["g"][:, 0:ND].rearrange("p (i d) -> p i d", i=NT)
                    nc.scalar.dma_start(out=cdst, in_=ctx_d[b])
                    v["q"] = spool.tile([J, D], BF16, tag="q", name=f"q{b}")
                    nc.scalar.dma_start(out=v["q"], in_=qry_d[b])
                    v["qT"] = spool.tile([128, NK * J], BF16, tag="qt", name=f"qT{b}")
                    nc.scalar.dma_start(out=v["qT"], in_=qryt_d[b])

                def stage_b(b):
                    v = st[b]
                    qT_sb, q_sb = v["qT"], v["q"]
                    qhatT = spool.tile([128, NK * J], BF16, tag="qhat", name=f"qhat{b}")
                    for k in range(NK):
                        nc.scalar.activation(
                            qhatT[:, J * k : J * (k + 1)],
                            qT_sb[:, J * k : J * (k + 1)],
                            ACTF.Identity,
                            bias=wt_sb[:, k : k + 1],
                            scale=wt_sb[:, 8 + k : 9 + k],
                        )
                    qt_ps2 = ps_s.tile([J, 1], F32, tag="s", name=f"qt_ps{b}")
                    for k in range(NK):
                        nc.tensor.matmul(
                            qt_ps2,
                            lhsT=qT_sb[:, J * k : J * (k + 1)],
                            rhs=wt_bf[:, 4 + k : 5 + k],
                            start=(k == 0),
                            stop=(k == NK - 1),
                        )
                    qt_col = small.tile([J, 1], F32, tag="qtc", name=f"qt_col{b}")
                    nc.scalar.copy(qt_col, qt_ps2)

                    ct_ps = ps_trans.tile([128, NK * T], BF16, tag="trans", name=f"ct_ps{b}")
                    for k in range(NK):
                        for i in range(NT):
                            nc.tensor.transpose(
                                ct_ps[:, T * k + 128 * i : T * k + 128 * (i + 1)],
                                gs(b, 0, i)[:, 128 * k : 128 * (k + 1)],
                                ident_bf,
                            )
                    ct_sb = spool.tile([128, NK * T], BF16, tag="ct", name=f"ct_sb{b}")
                    half = NK * T // 2
                    nc.vector.tensor_copy(ct_sb[:, 0:half], ct_ps[:, 0:half])
                    nc.scalar.copy(ct_sb[:, half:], ct_ps[:, half:])

                    st_ps = ps_s.tile([J, T], F32, tag="s", name=f"st_ps{b}")
                    for k in range(NK):
                        nc.tensor.matmul(
                            st_ps,
                            lhsT=qhatT[:, J * k : J * (k + 1)],
                            rhs=ct_sb[:, T * k : T * (k + 1)],
                            start=(k == 0),
                            stop=(k == NK - 1),
                        )
                    ptr_sb = spool.tile([J, T], BF16, tag="pt", name=f"ptr{b}")
                    nc.scalar.activation(
                        ptr_sb, st_ps, ACTF.Exp, bias=qt_col, scale=1.0
                    )
                    v["ptr"] = ptr_sb
                    pall_ps = ps_pall.tile([128, NT * J], BF16, tag="pall", name=f"pall{b}")
                    for i in range(NT):
                        nc.tensor.transpose(
                            pall_ps[:, J * i : J * (i + 1)],
                            ptr_sb[:, 128 * i : 128 * (i + 1)],
                            ident_bf[:J, :J],
                        )
                    pall_v = pall_ps.rearrange("p (i j) -> p i j", i=NT)
                    e_sb = small.tile([128, NT], BF16, tag="e", name=f"e{b}")
                    rs_sb = small.tile([128, NT], F32, tag="rs", name=f"rs{b}")
                    recip = small.tile([128, NT], F32, tag="rcp", name=f"recip{b}")
                    nc.vector.reduce_max(e_sb, pall_v, axis=mybir.AxisListType.X)
                    nc.vector.reduce_sum(rs_sb, pall_v, axis=mybir.AxisListType.X)
                    nc.vector.reciprocal(recip, rs_sb)
                    v["e"], v["recip"] = e_sb, recip

                def stage_c(b):
                    v = st[b]
                    q_sb, ptr_sb, e_sb, recip = v["q"], v["ptr"], v["e"], v["recip"]
                    q2c_ps = ps_q2c.tile([1, D], F32, tag="q2c", name=f"q2c_ps{b}")
                    se_ps = ps_se.tile([1, 1], F32, tag="se", name=f"se_ps{b}")
                    for i in range(NT):
                        c2q_ps = ps_mm.tile([128, D], F32, tag="mm", name=f"c2q{b}_{i}")
                        nc.tensor.matmul(
                            c2q_ps,
                            lhsT=ptr_sb[:, 128 * i : 128 * (i + 1)],
                            rhs=q_sb,
                            start=True,
                            stop=True,
                        )
                        if i % 2 == 0:
                            nc.scalar.activation(
                                gs(b, 1, i),
                                c2q_ps,
                                ACTF.Copy,
                                scale=recip[:, i : i + 1],
                            )
                        else:
                            nc.vector.tensor_scalar_mul(
                                gs(b, 1, i), c2q_ps, recip[:, i : i + 1]
                            )
                        eng = nc.gpsimd if i == 0 else nc.vector
                        eng.tensor_mul(gs(b, 2, i), gs(b, 1, i), gs(b, 0, i))
                        nc.tensor.matmul(
                            q2c_ps,
                            lhsT=e_sb[:, i : i + 1],
                            rhs=gs(b, 0, i),
                            start=(i == 0),
                            stop=(i == NT - 1),
                        )
                        nc.tensor.matmul(
                            se_ps,
                            lhsT=e_sb[:, i : i + 1],
                            rhs=ones_col_bf,
                            start=(i == 0),
                            stop=(i == NT - 1),
                        )
                    if not no_store:
                        nc.sync.dma_start(
                            out=out_d[b, :, 0:2],
                            in_=v["g"][:, ND : 3 * ND].rearrange(
                                "p (s i d) -> p s i d", s=2, i=NT
                            ),
                        )
                    rcp_s = small.tile([1, 1], F32, tag="rcps", name=f"rcp_s{b}")
                    nc.vector.reciprocal(rcp_s, se_ps)
                    q2c_row = small.tile([1, D], BF16, tag="q2cr", name=f"q2c_row{b}")
                    nc.scalar.activation(
                        q2c_row, q2c_ps, ACTF.Copy, scale=rcp_s
                    )
                    bc_ps = ps_mm.tile([128, D], F32, tag="mm", name=f"bc_ps{b}")
                    nc.tensor.matmul(
                        bc_ps, lhsT=ones_row_bf, rhs=q2c_row, start=True, stop=True
                    )
                    bc_sb = small.tile([128, D], BF16, tag="bc", name=f"bc_sb{b}")
                    nc.scalar.copy(bc_sb, bc_ps)
                    for i in range(NT):
                        eng = nc.gpsimd if i in (0, 2) else nc.vector
                        eng.tensor_mul(gs(b, 3, i), gs(b, 0, i), bc_sb)
                    if not no_store:
                        nc.sync.dma_start(
                            out=out_d[b, :, 2],
                            in_=v["g"][:, 3 * ND : 4 * ND].rearrange(
                                "p (i d) -> p i d", i=NT
                            ),
                        )

                if dma_only:
                    for b in range(BPC):
                        stage_a(b)
                        v = st[b]
                        nc.sync.dma_start(
                            out=out_d[b, :, 0:2],
                            in_=v["g"][:, 0 : 2 * ND].rearrange(
                                "p (s i d) -> p s i d", s=2, i=NT
                            ),
                        )
                        nc.scalar.dma_start(
                            out=out_d[b, :, 2],
                            in_=v["g"][:, 0:ND].rearrange("p (i d) -> p i d", i=NT),
                        )
                else:
                    plan = []
                    for b in range(BPC + 2):
                        if b < BPC:
                            plan.append(("A", b))
                        if 1 <= b < BPC + 1:
                            plan.append(("B", b - 1))
                        if b >= 2:
                            plan.append(("C", b - 2))
                    for s, b in plan:
                        {"A": stage_a, "B": stage_b, "C": stage_c}[s](b)

    nc.compile()
    return nc


_NC_CACHE = None


def _get_nc():
    global _NC_CACHE
    if _NC_CACHE is None:
        _NC_CACHE = build_kernel()
    return _NC_CACHE


def _prep_in_maps(embd_context, embd_query, W):
    bf = ml_dtypes.bfloat16
    c_bf = np.asarray(embd_context, dtype=np.float32).astype(bf)
    q_bf = np.asarray(embd_query, dtype=np.float32).astype(bf)
    c_resh = np.ascontiguousarray(
        c_bf.reshape(B, NT, 128, D).transpose(0, 2, 1, 3)
    )
    qt_resh = np.ascontiguousarray(
        q_bf.transpose(0, 2, 1)
        .reshape(B, NK, 128, J)
        .transpose(0, 2, 1, 3)
        .reshape(B, 128, NK * J)
    )
    w_resh = np.ascontiguousarray(
        np.asarray(W, dtype=np.float32).reshape(12, 128).T
    )
    in_maps = []
    for c in range(N_CORES):
        sl = slice(c * BPC, (c + 1) * BPC)
        in_maps.append(
            {
                "c_resh": np.ascontiguousarray(c_resh[sl]),
                "embd_query": np.ascontiguousarray(q_bf[sl]),
                "qt_resh": np.ascontiguousarray(qt_resh[sl]),
                "w_resh": w_resh,
            }
        )
    return in_maps


def run_spmd(embd_context, embd_query, W, **spmd_kwargs):
    nc = _get_nc()
    in_maps = _prep_in_maps(embd_context, embd_query, W)
    res = run_bass_kernel_spmd(nc, in_maps, core_ids=list(range(N_CORES)), **spmd_kwargs)
    out = np.empty((B, T, GD), np.float32)
    out[:, :, :D] = np.asarray(embd_context, np.float32)
    out[:, :, D:] = np.concatenate(
        [
            res.results[c]["g_out"].transpose(0, 3, 1, 2, 4).reshape(BPC, T, 3 * D)
            for c in range(N_CORES)
        ],
        axis=0,
    ).astype(np.float32)
    return out, res


def kernel(embd_context, embd_query, W):
    out, _ = run_spmd(embd_context, embd_query, W)
    return out
